# revision 16
# baseline (speedup 1.0000x reference)
"""Dense 2-layer GAT (4 heads) on 8 Trainium2 NeuronCores — v2.

Row-parallel over destination nodes (512 rows/core). Rewritten from the
f32 baseline around three ideas:

1. bf16 everywhere in the hot path: adjacency mask, projections, and
   attention tiles. Halves DMA bytes, 2x/4x DVE modes, ~4x faster
   matmuls (1 cycle/col vs 4 for f32).

2. Rank-1 attention construction: exp is monotonic, so
     exp(lrelu(e)) = max(exp(e), exp(0.2 e)),  e[j,i] = d_j + s_i
   and both branches factor: exp(e) = B_j * A_i with A = exp(s),
   B = exp(d) (per-node exps only — the dense [N, R] tiles never touch
   the Activation engine on this path):
     v   = a_rep * b_j          (DVE tensor_scalar, 4x mode)
     m   = max(A_rep * B_j, v)  (DVE/GpSimd scalar_tensor_tensor)
     att = m * adjT             (DVE/GpSimd tensor_tensor, exact mask)
   A ScalarE flavor (add + Prelu + Exp) is kept as well and work is
   spread across DVE / Act / GpSimd per (layer, head, superblock) via a
   static strategy table.

3. adjT is DMA'd once (bf16, 4.2 MB) and stays in SBUF for both layers.

Host-side prep is layout/dtype only, plus folding a_dst into the weight
matrices (wtld = W @ a_dst) so the front matmuls emit Wh and d together.
"""
import sys

if "/opt/trn_rl_repo" not in sys.path:
    sys.path.insert(0, "/opt/trn_rl_repo")

import numpy as np

import concourse.bacc as bacc
import concourse.mybir as mybir
import concourse.tile as tile
from concourse.bass_utils import run_bass_kernel_spmd

F32 = mybir.dt.float32
BF16 = mybir.dt.bfloat16
AF = mybir.ActivationFunctionType
OP = mybir.AluOpType

N = 4096
NFEAT = 512
NHID = 256
NEMBED = 128
NHEADS = 4
O1 = 64
O2 = 32
NCORES = 8
R = N // NCORES          # 512 rows per core
ALPHA = 0.2
NT = N // 128            # 32 j-tiles
SB = 8                   # j-tiles per superblock
NSB = NT // SB           # 4 superblocks
B1 = O1 + 1              # 65-col block per head in WhD1 [Wh_h | ones]
B2 = O2 + 1              # 33
G2 = NHEADS * B2         # 132: layer-2 gather cols [Wh2_h | d2_h] x 4


def _default_table():
    """(layer, head, sb) -> strategy.

    xd : rank-1, all on DVE        (ts + stt-max + tt-mask)
    xg : rank-1, max on GpSimd     (DVE ts/mask, gp stt-max)
    xgm: rank-1, max+mask on GpSimd
    y  : ScalarE (DVE add, Act Prelu+Exp, DVE mask)
    yp : ScalarE, mask on GpSimd
    """
    tbl = {}
    pats = [["xg", "ya", "xm", "y"],
            ["y", "xg", "yp", "xg"],
            ["xm", "y", "xg", "ya"],
            ["ya", "xm", "xg", "yp"]]
    for l in (1, 2):
        for sb in range(NSB):
            for h in range(NHEADS):
                tbl[(l, h, sb)] = pats[sb][h]
    return tbl


VARIANT = {}
if __import__("os").environ.get("BASS_VARIANT"):
    VARIANT.update(__import__("json").loads(
        __import__("os").environ["BASS_VARIANT"]))


def _on(flag):
    return VARIANT.get(flag, False)


def _table():
    if VARIANT.get("table_all"):
        return {(l, h, sb): VARIANT["table_all"]
                for l in (1, 2) for h in range(NHEADS) for sb in range(NSB)}
    if VARIANT.get("pats"):
        pats = VARIANT["pats"]
        return {(l, h, sb): pats[sb][h]
                for l in (1, 2) for h in range(NHEADS) for sb in range(NSB)}
    return VARIANT.get("table") or _default_table()


def _build(debug=False, repeat=1):
    nc = bacc.Bacc("TRN2", target_bir_lowering=False, debug=False,
                   num_devices=NCORES)

    xT = nc.dram_tensor("xT", [NFEAT, N], BF16, kind="ExternalInput").ap()
    xmT = nc.dram_tensor("xmT", [NFEAT, R], BF16, kind="ExternalInput").ap()
    adjT = nc.dram_tensor("adjT", [N, R], BF16, kind="ExternalInput").ap()
    wr1 = nc.dram_tensor("wr1", [NFEAT, NHEADS * B1], BF16,
                         kind="ExternalInput").ap()
    asrc1 = nc.dram_tensor("asrc1", [1, NHEADS * O1], F32,
                           kind="ExternalInput").ap()
    w2r = nc.dram_tensor("w2r", [NHID, G2], BF16, kind="ExternalInput").ap()
    asrc2 = nc.dram_tensor("asrc2", [1, NHEADS * O2], F32,
                           kind="ExternalInput").ap()
    out = nc.dram_tensor("h2T", [NEMBED, R], F32, kind="ExternalOutput").ap()

    with tile.TileContext(nc) as tc:
        for _rep in range(repeat):
            _emit(tc, nc, xT, xmT, adjT, wr1, asrc1, w2r, asrc2, out)
    nc.compile()
    return nc


def _emit(tc, nc, xT, xmT, adjT, wr1, asrc1, w2r, asrc2, out):
    v_ = nc.vector
    s_ = nc.scalar
    t_ = nc.tensor
    g_ = nc.gpsimd
    tbl = _table()

    def needs(l, h, kinds):
        return any(tbl[(l, h, sb)] in kinds for sb in range(NSB))

    XK = ("xd", "xg", "xm", "xgm")
    YK = ("y", "ya", "yp")

    with (
        tc.tile_pool(name="persist", bufs=1) as P,
        tc.tile_pool(name="small", bufs=2) as SP,
        tc.tile_pool(name="psA", bufs=1, space="PSUM") as PSA,
        tc.tile_pool(name="psB", bufs=VARIANT.get("psb_bufs", 3),
                     space="PSUM") as PSB,
        tc.tile_pool(name="vj", bufs=VARIANT.get("vj_bufs", 20)) as VPJ,
        tc.tile_pool(name="vh", bufs=VARIANT.get("vh_bufs", 8)) as VPH,
        tc.tile_pool(name="dram", bufs=1, space="DRAM") as DP,
    ):
        ones_row = P.tile([1, 128], F32, tag="ones_row")
        v_.memset(ones_row[:], 1.0)

        # ---- a_src replicated [Fo, 128] per head, bf16 ------------------
        asrc_rep = {}
        for l, af, Fo in ((1, asrc1, O1), (2, asrc2, O2)):
            for h in range(NHEADS):
                col = P.tile([Fo, 1], F32, tag=f"asc_{l}_{h}")
                nc.sync.dma_start(col[:], af[0:1, Fo * h: Fo * h + Fo])
                rep = P.tile([Fo, 128], BF16, tag=f"asr_{l}_{h}")
                v_.memset(rep[:], 0.0)
                s_.activation(rep[:], rep[:], AF.Identity, bias=col[:],
                              scale=0.0)
                asrc_rep[(l, h)] = rep

        # ---- persistent adjacency (both layers) -------------------------
        AdjT = P.tile([128, NT, R], BF16, tag="AdjT")
        for b in range(NSB):
            nc.scalar.dma_start(
                AdjT[:, SB * b: SB * b + SB, :],
                adjT[SB * 128 * b: SB * 128 * (b + 1), :].rearrange(
                    "(t p) i -> p t i", p=128))

        WR1 = P.tile([128, 4, NHEADS * B1], BF16, tag="WR1")
        nc.sync.dma_start(WR1[:], wr1[:, :].rearrange("(c p) o -> p c o",
                                                      p=128))
        XM = P.tile([128, 4, R], BF16, tag="XM")
        nc.sync.dma_start(XM[:], xmT[:, :].rearrange("(c p) n -> p c n",
                                                     p=128))
        W2R = P.tile([128, 2, G2], BF16, tag="W2R")
        nc.sync.dma_start(W2R[:], w2r[:, :].rearrange("(c p) o -> p c o",
                                                      p=128))

        # ---- s_rep / A_rep / a_rep for layer 1 (my rows) ----------------
        reps1 = _make_reps(nc, P, SP, PSB, asrc_rep, WR1, XM, 1, O1, B1,
                           tbl, needs, XK, YK)

        # ---- layer-1 front: WhD1 + D1 -----------------------------------
        WhD1 = P.tile([128, NT, NHEADS * B1], BF16, tag="WhD1")
        D1 = P.tile([128, NT, NHEADS], F32, tag="D1")
        Be1 = P.tile([128, NT, NHEADS], F32, tag="Be1")
        be1 = P.tile([128, NT, NHEADS], F32, tag="be1")
        XT = P.tile([128, 4, N // 2], BF16, tag="XT")
        ntph = NT // 2
        for half in range(2):
            for q in range(4):
                w = N // 8
                nc.sync.dma_start(
                    XT[:, :, w * q: w * (q + 1)],
                    xT[:, (N // 2) * half + w * q:
                       (N // 2) * half + w * (q + 1)].rearrange(
                        "(c p) n -> p c n", p=128))
            hs = slice(ntph * half, ntph * (half + 1))
            for nt in range(ntph * half, ntph * half + ntph):
                ntl = nt - ntph * half
                fps = PSB.tile([128, NHEADS * B1], F32, tag="ps")
                for fc in range(4):
                    t_.matmul(fps[:], XT[:, fc, 128 * ntl: 128 * ntl + 128],
                              WR1[:, fc, :], start=(fc == 0), stop=(fc == 3))
                s_.copy(WhD1[:, nt, :], fps[:])
                v_.tensor_copy(D1[:, nt, :], fps[:, O1: NHEADS * B1: B1])
            # overwrite the wtld column with ones for the denominator
            v_.memset(WhD1[:, hs, O1: NHEADS * B1: B1], 1.0)
            s_.activation(Be1[:, hs, :], D1[:, hs, :], AF.Exp)
            s_.activation(be1[:, hs, :], D1[:, hs, :], AF.Exp,
                          bias=0.0, scale=ALPHA)

        # ---- layer-1 attention + tails ----------------------------------
        hT = [PSA.tile([B1, R], F32, tag=f"hT_{h}", name=f"hT1_{h}")
              for h in range(NHEADS)]
        H1T = P.tile([128, 2, R], BF16, tag="H1T")
        _att_layer(nc, tbl, 1, WhD1, D1, Be1, be1, reps1, AdjT, hT,
                   VPJ, VPH, B1, O1)
        for h in range(NHEADS):
            _tail(nc, SP, PSB, ones_row, hT[h], H1T, 1, O1, h)

        # ---- layer-2 front: Gsb = [Wh2_h | d2_h] for my rows ------------
        Gsb = SP.tile([128, 4, G2], BF16, tag="Gsb")
        for it in range(4):
            gps = PSB.tile([128, G2], F32, tag="ps")
            for fc in range(2):
                t_.matmul(gps[:], H1T[:, fc, 128 * it: 128 * it + 128],
                          W2R[:, fc, :], start=(fc == 0), stop=(fc == 1))
            v_.tensor_copy(Gsb[:, it, :], gps[:])

        reps2 = _make_reps(nc, P, SP, PSB, asrc_rep, W2R, H1T, 2, O2, B2,
                           tbl, needs, XK, YK)

        Gmine = DP.tile([R, G2], BF16, tag="Gmine")
        Gall = DP.tile([N, G2], BF16, tag="Gall", addr_space="Shared")
        nc.sync.dma_start(
            Gmine[:, :].rearrange("(t p) o -> p t o", p=128), Gsb[:])
        if _on("skip_gather"):
            nc.sync.dma_start(Gall[0:R, :], Gmine[:, :])
        else:
            nc.gpsimd.collective_compute(
                "AllGather", OP.bypass,
                replica_groups=[list(range(NCORES))],
                ins=[Gmine[:].opt()], outs=[Gall[:].opt()],
            )

        WhD2 = P.tile([128, NT, G2], BF16, tag="WhD2")
        for q in range(4):
            tw = NT // 4
            nc.sync.dma_start(
                WhD2[:, tw * q: tw * (q + 1), :],
                Gall[128 * tw * q: 128 * tw * (q + 1), :].rearrange(
                    "(t p) o -> p t o", p=128))
        D2 = P.tile([128, NT, NHEADS], F32, tag="D2")
        v_.tensor_copy(D2[:], WhD2[:, :, O2: G2: B2])
        v_.memset(WhD2[:, :, O2: G2: B2], 1.0)
        Be2 = P.tile([128, NT, NHEADS], F32, tag="Be2")
        be2 = P.tile([128, NT, NHEADS], F32, tag="be2")
        s_.activation(Be2[:], D2[:], AF.Exp)
        s_.activation(be2[:], D2[:], AF.Exp, bias=0.0, scale=ALPHA)

        # ---- layer-2 attention + tails ----------------------------------
        hT2 = [PSA.tile([B1, R], F32, tag=f"hT_{h}", name=f"hT2_{h}")
               for h in range(NHEADS)]
        H2T = P.tile([NEMBED, R], F32, tag="H2T")
        _att_layer(nc, tbl, 2, WhD2, D2, Be2, be2, reps2, AdjT,
                   [t[0:B2, :] for t in hT2], VPJ, VPH, B2, O2)
        for h in range(NHEADS):
            _tail(nc, SP, PSB, ones_row, hT2[h][0:B2, :], H2T, 2, O2, h)

        nc.sync.dma_start(out[:, :], H2T[:])


def _make_reps(nc, P, SP, PSB, asrc_rep, WRt, XMt, l, Fo, blk, tbl, needs,
               XK, YK):
    """Per-head [128, R] tiles: s_rep (bf16), A_rep=exp(s), a_rep=exp(.2 s).

    wps = Wh_mine^T via WRt x XMt; sps = asrc_rep x wsb gives s replicated
    across partitions; Act converts psum to the needed flavors.
    """
    v_ = nc.vector
    s_ = nc.scalar
    t_ = nc.tensor
    nfc = 4 if l == 1 else 2
    reps = {}
    for h in range(NHEADS):
        wps = PSB.tile([Fo, R], F32, tag="ps", name=f"wps{l}_{h}")
        for fc in range(nfc):
            t_.matmul(wps[:], WRt[:, fc, blk * h: blk * h + Fo],
                      XMt[:, fc, :], start=(fc == 0), stop=(fc == nfc - 1))
        wsb = SP.tile([Fo, R], BF16, tag="wsb", name=f"wsb{l}_{h}")
        v_.tensor_copy(wsb[:], wps[:])
        sps = PSB.tile([128, R], F32, tag="ps", name=f"sps{l}_{h}")
        t_.matmul(sps[:], asrc_rep[(l, h)][:], wsb[:], start=True, stop=True)
        ent = {}
        if needs(l, h, ("xd", "xg", "xm", "xgm")):
            Ar = P.tile([128, R], BF16, tag=f"Ar{l}_{h}")
            s_.activation(Ar[:], sps[:], AF.Exp)
            ar = P.tile([128, R], BF16, tag=f"ar{l}_{h}")
            s_.activation(ar[:], sps[:], AF.Exp, bias=0.0, scale=ALPHA)
            ent["A"], ent["a"] = Ar, ar
        if needs(l, h, ("xc",)):
            # att = A_i * max(B_j, b_j * c_i) * adj with c = exp(-0.8 s);
            # the A_i factor cancels between numerator and denominator.
            cr = P.tile([128, R], BF16, tag=f"cr{l}_{h}")
            s_.activation(cr[:], sps[:], AF.Exp, bias=0.0, scale=ALPHA - 1.0)
            ent["c"] = cr
        if needs(l, h, YK):
            sr = P.tile([128, R], BF16, tag=f"sr{l}_{h}")
            s_.activation(sr[:], sps[:], AF.Identity)
            ent["s"] = sr
        reps[h] = ent
    return reps


def _att_layer(nc, tbl, l, WhD, D, Be, be, reps, AdjT, hT, VPJ, VPH, blk, Fo):
    v_ = nc.vector
    s_ = nc.scalar
    t_ = nc.tensor
    g_ = nc.gpsimd
    HW = SB // 2  # 4-jt wave
    with nc.named_scope(f"att_l{l}"):
        for b in range(NSB):
            for q in range(2):
                j0 = SB * b + HW * q
                vt = {}
                # stage 1: rank-1 v (DVE or Pool ts) / ScalarE adds
                for h in range(NHEADS):
                    code = tbl[(l, h, b)]
                    ent = reps[h]
                    if code[0] == "x":
                        vt[h] = [VPJ.tile([128, R], BF16, tag="vj",
                                          name=f"vj_{l}_{b}_{q}_{h}_{t}")
                                 for t in range(HW)]
                        src = ent["c"] if code == "xc" else ent["a"]
                        veng = g_ if code in ("xg", "xgm") else v_
                        for t in range(HW):
                            veng.tensor_scalar(vt[h][t][:], src[:],
                                               be[:, j0 + t, h: h + 1], None,
                                               op0=OP.mult)
                    else:
                        vh = VPH.tile([128, HW, R], BF16, tag="vh")
                        vt[h] = vh
                        if code == "ya":
                            continue  # add folded into Prelu bias below
                        for t in range(HW):
                            v_.tensor_scalar(vh[:, t, :], ent["s"][:],
                                             D[:, j0 + t, h: h + 1], None,
                                             op0=OP.add)
                # stage 2: max(u, v) on DVE stt, Prelu+Exp on Act
                for h in range(NHEADS):
                    code = tbl[(l, h, b)]
                    ent = reps[h]
                    if code == "xc":
                        for t in range(HW):
                            v_.tensor_scalar(vt[h][t][:], vt[h][t][:],
                                             Be[:, j0 + t, h: h + 1], None,
                                             op0=OP.max)
                    elif code[0] == "x":
                        if _on("no_stt"):
                            for t in range(HW):
                                u = VPJ.tile([128, R], BF16, tag="vj",
                                             name=f"uj_{l}_{b}_{q}_{h}_{t}")
                                v_.tensor_scalar(u[:], ent["A"][:],
                                                 Be[:, j0 + t, h: h + 1],
                                                 None, op0=OP.mult)
                                v_.tensor_tensor(vt[h][t][:], vt[h][t][:],
                                                 u[:], op=OP.max)
                        else:
                            for t in range(HW):
                                v_.scalar_tensor_tensor(
                                    vt[h][t][:], ent["A"][:],
                                    Be[:, j0 + t, h: h + 1],
                                    vt[h][t][:], op0=OP.mult, op1=OP.max)
                    elif code == "ya":
                        vh = vt[h]
                        for t in range(HW):
                            s_.activation(vh[:, t, :], ent["s"][:], AF.Prelu,
                                          bias=D[:, j0 + t, h: h + 1],
                                          scale=1.0, alpha=ALPHA)
                        s_.activation(vh[:], vh[:], AF.Exp)
                    else:
                        vh = vt[h]
                        s_.activation(vh[:], vh[:], AF.Prelu, bias=0.0,
                                      scale=1.0, alpha=ALPHA)
                        s_.activation(vh[:], vh[:], AF.Exp)
                # stage 3: mask; stage 4: aggregation matmuls
                for h in range(NHEADS):
                    code = tbl[(l, h, b)]
                    if code[0] == "x":
                        keng = g_ if code in ("xm", "xgm") else v_
                        for t in range(HW):
                            keng.tensor_tensor(vt[h][t][:], vt[h][t][:],
                                               AdjT[:, j0 + t, :],
                                               op=OP.mult)
                    else:
                        meng = g_ if code == "yp" else v_
                        meng.tensor_tensor(vt[h][:], vt[h][:],
                                           AdjT[:, j0: j0 + HW, :],
                                           op=OP.mult)
                for h in range(NHEADS):
                    code = tbl[(l, h, b)]
                    vs = vt[h]
                    for t in range(HW):
                        jt = j0 + t
                        src = vs[t][:] if isinstance(vs, list) else vs[:, t, :]
                        t_.matmul(hT[h], WhD[:, jt, blk * h: blk * h + blk],
                                  src, start=(jt == 0), stop=(jt == NT - 1))


def _tail(nc, SP, PSB, ones_row, hTh, Hout, l, Fo, h):
    v_ = nc.vector
    s_ = nc.scalar
    t_ = nc.tensor
    r1 = SP.tile([1, R], F32, tag="recip", name=f"r1_{l}_{h}")
    v_.reciprocal(r1[:], hTh[Fo: Fo + 1, :])
    rps = PSB.tile([128, R], F32, tag="ps", name=f"rps_{l}_{h}")
    t_.matmul(rps[:], ones_row[:], r1[:], start=True, stop=True)
    odt = BF16 if l == 1 else F32
    rrep = SP.tile([128, R], F32, tag="rrep", name=f"rrep_{l}_{h}")
    v_.tensor_copy(rrep[:], rps[:])
    hn = SP.tile([Fo, R], odt, tag="hn", name=f"hn_{l}_{h}")
    v_.tensor_tensor(hn[:], hTh[0:Fo, :], rrep[0:Fo, :], op=OP.mult)
    # ELU(x) = max(x,0) - 1 + exp(min(x,0))
    m = SP.tile([Fo, R], odt, tag="elu_m", name=f"m_{l}_{h}")
    v_.tensor_scalar(m[:], hn[:], 0.0, None, op0=OP.min)
    s_.activation(m[:], m[:], AF.Exp)
    rl = SP.tile([Fo, R], odt, tag="elu_rl", name=f"rl_{l}_{h}")
    v_.tensor_scalar(rl[:], hn[:], 0.0, -1.0, op0=OP.max, op1=OP.add)
    if l == 1:
        dst = Hout[O1 * (h % 2): O1 * (h % 2) + O1, h // 2, :]
    else:
        dst = Hout[O2 * h: O2 * h + O2, :]
    v_.tensor_tensor(dst, m[:], rl[:], op=OP.add)


_NC_CACHE = {}


def _get_nc():
    if "nc" not in _NC_CACHE:
        _NC_CACHE["nc"] = _build()
    return _NC_CACHE["nc"]


def _in_maps_for(inputs):
    import ml_dtypes
    BF = ml_dtypes.bfloat16
    x = np.asarray(inputs["x"], dtype=np.float32)
    adj = np.asarray(inputs["adj"], dtype=np.float32)
    W1 = np.asarray(inputs["W1"], np.float32)
    a1 = np.asarray(inputs["a1"], np.float32)
    W2 = np.asarray(inputs["W2"], np.float32)
    a2 = np.asarray(inputs["a2"], np.float32)

    xT = np.ascontiguousarray(x.T).astype(BF)
    wr1 = np.zeros((NFEAT, NHEADS * B1), np.float32)
    asrc1 = np.zeros((1, NHEADS * O1), np.float32)
    for h in range(NHEADS):
        wr1[:, B1 * h: B1 * h + O1] = W1[h]
        wr1[:, B1 * h + O1] = W1[h] @ a1[h, O1:]
        asrc1[0, O1 * h: O1 * h + O1] = a1[h, :O1]
    w2r = np.zeros((NHID, G2), np.float32)
    asrc2 = np.zeros((1, NHEADS * O2), np.float32)
    for h in range(NHEADS):
        w2r[:, B2 * h: B2 * h + O2] = W2[h]
        w2r[:, B2 * h + O2] = W2[h] @ a2[h, O2:]
        asrc2[0, O2 * h: O2 * h + O2] = a2[h, :O2]
    wr1 = np.ascontiguousarray(wr1).astype(BF)
    w2r = np.ascontiguousarray(w2r).astype(BF)

    in_maps = []
    for c in range(NCORES):
        rows = slice(R * c, R * (c + 1))
        in_maps.append({
            "xT": xT,
            "xmT": np.ascontiguousarray(x[rows, :].T).astype(BF),
            "adjT": np.ascontiguousarray(adj[rows, :].T).astype(BF),
            "wr1": wr1, "asrc1": asrc1, "w2r": w2r, "asrc2": asrc2,
        })
    return in_maps


def kernel(x, adj, W1, a1, W2, a2):
    nc = _get_nc()
    in_maps = _in_maps_for(dict(x=x, adj=adj, W1=W1, a1=a1, W2=W2, a2=a2))
    res = run_bass_kernel_spmd(nc, in_maps, core_ids=list(range(NCORES)))
    return np.concatenate(
        [np.asarray(res.results[c]["h2T"]).T for c in range(NCORES)], axis=0)
